# revision 12
# baseline (speedup 1.0000x reference)
"""Trainium2 Bass kernel v2 for nn_InterleavedHiddenMarkovChain_47261820125822.

Same exact collapsed math as the baseline (see kernel.py docstring), but
re-engineered against the trace evidence from the 66us baseline:

 1. Exp/Ln ONLY (they share one activation table: `natural_log_exp_and_others`)
    -> one hidden ACT_TABLE_LOAD instead of 12 (-15.4us).
 2. Observation-gather formulation: the host gathers emis[:, ys] (pure int
    indexing -> "index prep"), so the time-sum becomes a plain innermost-axis
    tensor_reduce over t in a (48 x 48 x 64) view -- no fp32 PE reduction, no
    single-partition (1,2304) ops (-~29us).
 3. The per-s1 normalization term rides into PSUM via cheap rank-1 f32r
    matmuls pre-accumulated with the emission rows; the big pass is a
    6-chunk DVE->ACT->ACT->DVE pipeline over (48p, 512f) tiles.
 4. All partition-crossing is done by tiny PE transposes/rank-1 matmuls.

Math recap (K=2, S=48, A=64, T=64):
    ghat_c[s,a] = choice_l[c] + trans_l[c,s,s] + emis_l[c,s,a]
    beta[s0,s1] = prior_l[0,s0] + prior_l[1,s1]
                  + sum_t LSE(ghat_0[s0,y_t], ghat_1[s1,y_t])
    answer      = LSE_{s0,s1} beta     (+ LSE_c(choice_l) == 0 exactly)
with LSE(p,q) = q + ln1p(exp(p-q)):
    sum_t LSE(...) = R1h[s1] + W[s0,s1]
    R1h[s1] = sum_t emis[1,s1,y_t] + T*(PC1[s1] + cl1)
    W[s0,s1] = sum_t ln1p(exp(D[s0,s1,t]))
    D[s0,s1,t] = g0G[s0,t] - (emis[1,s1,y_t] + PC1[s1])
    g0G[s0,t] = emis[0,s0,y_t] + PC0[s0] + (cl0-cl1)
    PCc[s] = trans[c,s,s] - lseT[c,s] - lseE[c,s]   (c-row logsumexps)
"""

import numpy as np

import concourse.bass as bass
import concourse.bacc as bacc
import concourse.mybir as mybir
from concourse import tile
from concourse.bass_utils import run_bass_kernel_spmd

F32 = mybir.dt.float32
F32R = mybir.dt.float32r
AF = mybir.ActivationFunctionType
AX = mybir.AxisListType
OP = mybir.AluOpType

K, S, A, T = 2, 48, 64, 64
N_CORES = 8
NEG = -1.0e30

# PK column layout (128 partitions; c0 rows at 0:48, c1 rows at 64:112)
C_TT = 0          # 48: transition rows
C_EM = 48         # 64: emission rows
C_DG = 112        # 1:  transition diagonal
C_EMG = 113       # 64: emis[c][:, ys] gather
C_PRI = 177       # 1:  prior[c] as column
C_ID = 178        # 112: identity(112)
PK_W = 292

# FLAT row layout (single partition)
F_PRI0 = 0        # 48
F_PRI1 = 48       # 48
F_CH = 96         # 2 (then -1e30 pad to 144)
F_ONES = 144      # 48
F_EMF = 192       # 3072: emis[1, s1, ys[t]] at s1*64+t
FLAT_W = F_EMF + S * T   # 3264

NCHUNK = 3
CW = (S * T) // NCHUNK   # 1024 free per chunk = 16 s1-values x 64 t
S1C = S // NCHUNK        # 16
HALF = CW // 2           # psum-bank-sized matmul half

_CACHED_NC = None

# The stock table-load pass serves Exp from `exp_and_others` and Ln from
# `natural_log` (first table containing each), reloading the activation
# table on every Exp<->Ln alternation (1283ns each, 13 loads in this
# kernel).  Both live in `natural_log_exp_and_others`; restrict the
# analysis (not the emitted ids -- dict order is preserved) so that is
# the only table serving them -> exactly one load, hoisted to the top.
_SHARED_TAB = "natural_log_exp_and_others"
_orig_get_tables = bacc.get_activation_tables


def _lnexp_tables(arch):
    out = {}
    for name, funcs in _orig_get_tables(arch).items():
        if name != _SHARED_TAB:
            funcs = funcs - {AF.Exp, AF.Ln}
        out[name] = funcs
    return out


bacc.get_activation_tables = _lnexp_tables


def _build_nc():
    nc = bacc.Bacc("TRN2", target_bir_lowering=False, debug=False)

    pk_d = nc.dram_tensor("pk", [128, PK_W], F32, kind="ExternalInput")
    fl_d = nc.dram_tensor("flat", [1, FLAT_W], F32R, kind="ExternalInput")
    out_d = nc.dram_tensor("out", [1, 1], F32, kind="ExternalOutput")

    with tile.TileContext(nc) as tc:
        with (
            tc.tile_pool(name="sb", bufs=1) as sb,
            tc.tile_pool(name="ps", bufs=1, space="PSUM") as ps,
        ):
            PK = sb.tile([128, PK_W], F32, tag="PK")
            nc.sync.dma_start(PK[:], pk_d[:, :])
            FL = sb.tile([1, FLAT_W], F32R, tag="FL")
            nc.sync.dma_start(FL[:], fl_d[:, :])

            ones48 = FL[:, F_ONES:F_ONES + 48]          # (1,48) f32r
            ones48f = ones48.bitcast(F32)

            # ---- batched row logsumexps --------------------------------
            NM = sb.tile([112, 8], F32, tag="NM")
            nc.vector.memset(NM[:], 0.0)
            SS = sb.tile([112, 8], F32, tag="SS")
            nc.vector.memset(SS[:], 1.0)

            nc.vector.tensor_reduce(NM[:, 0:1], PK[0:112, C_TT:C_EM + A],
                                    axis=AX.X, op=OP.max, negate=True)
            # prior0 | prior1 | choice(+pad) as 3x48 segments on partition 0
            nc.vector.tensor_reduce(
                NM[0:1, 2:5],
                FL[:, 0:144].bitcast(F32).rearrange("p (g w) -> p g w", g=3, w=48),
                axis=AX.X, op=OP.max, negate=True)

            # shared row max for the TT and EM segments (inputs are ~N(0,1),
            # so exp(x - m) cannot underflow for the smaller segment)
            EX = sb.tile([112, S + A], F32, tag="EX")
            nc.scalar.activation(EX[:], PK[0:112, C_TT:C_EM + A], AF.Exp,
                                 bias=NM[:, 0:1])
            nc.vector.tensor_reduce(SS[:, 0:1], EX[:, 0:S], axis=AX.X,
                                    op=OP.add)
            nc.vector.tensor_reduce(SS[:, 1:2], EX[:, S:S + A], axis=AX.X,
                                    op=OP.add)

            PRD = sb.tile([1, 144], F32, tag="PRD")
            nc.vector.tensor_tensor(
                PRD[:].rearrange("p (g w) -> p g w", g=3, w=48),
                FL[:, 0:144].bitcast(F32).rearrange("p (g w) -> p g w", g=3, w=48),
                NM[0:1, 2:5].unsqueeze(2).broadcast_to([1, 3, 48]),
                op=OP.add)
            E3 = sb.tile([1, 144], F32, tag="E3")
            nc.scalar.activation(E3[:], PRD[:], AF.Exp)
            nc.vector.tensor_reduce(
                SS[0:1, 2:5],
                E3[:].rearrange("p (g w) -> p g w", g=3, w=48),
                axis=AX.X, op=OP.add)

            LNS = sb.tile([112, 8], F32, tag="LNS")
            nc.scalar.activation(LNS[0:112, 0:5], SS[0:112, 0:5], AF.Ln)
            LSE = sb.tile([112, 8], F32, tag="LSE")
            nc.vector.tensor_tensor(LSE[0:1, 2:5], LNS[0:1, 2:5],
                                    NM[0:1, 2:5], op=OP.subtract)
            # LSE row0: 2=lseP0, 3=lseP1, 4=lseC

            # ---- per-row constants -------------------------------------
            # PCD = DG - lseT - lseE = DG - ln(S1) - ln(S2) + 2*nm
            T1 = sb.tile([112, 1], F32, tag="T1")
            nc.vector.tensor_tensor(T1[:], PK[0:112, C_DG:C_DG + 1],
                                    LNS[0:112, 0:1], op=OP.subtract)
            T2 = sb.tile([112, 1], F32, tag="T2")
            nc.vector.tensor_tensor(T2[:], T1[:], LNS[0:112, 1:2],
                                    op=OP.subtract)
            PCD = sb.tile([112, 1], F32, tag="PCD")
            nc.vector.tensor_scalar(PCD[:], NM[:, 0:1], 2.0, T2[:],
                                    op0=OP.mult, op1=OP.add)

            CL = sb.tile([1, 2], F32, tag="CL")
            nc.vector.tensor_scalar_sub(CL[:], FL[:, F_CH:F_CH + 2].bitcast(F32),
                                        LSE[0:1, 4:5])
            DCL = sb.tile([1, 1], F32, tag="DCL")
            nc.vector.tensor_tensor(DCL[:], CL[:, 0:1], CL[:, 1:2],
                                    op=OP.subtract)

            # ---- PE: transpose PCD -> row; rank-1 partition broadcasts --
            # all small PSUM tensors share one bank-sized tile (disjoint cols)
            SM = ps.tile([112, 512], F32, tag="sm")
            PCT_p = SM[0:1, 0:112]
            PSB0 = SM[0:48, 112:114]
            PSB1 = SM[64:112, 114:115]
            ZT_p = SM[0:1, 128:176]
            ZR_p = SM[0:48, 192:240]
            LT_p = SM[0:1, 256:304]
            nc.tensor.transpose(PCT_p, PCD[:], PK[0:112, C_ID:C_ID + 112])
            PCTs = sb.tile([1, 112], F32R, tag="PCTs")
            nc.vector.tensor_copy(PCTs[:], PCT_p)

            nc.tensor.matmul(PSB0[:, 0:1], ones48f, LSE[0:1, 2:3],
                             start=True, stop=True)
            nc.tensor.matmul(PSB0[:, 1:2], ones48f, DCL[:], start=True,
                             stop=True)
            nc.tensor.matmul(PSB1[:, 0:1], ones48f, LSE[0:1, 3:4],
                             start=True, stop=True)

            # ---- g0G and s1-side columns -------------------------------
            PCD0 = sb.tile([48, 1], F32, tag="PCD0")
            nc.vector.tensor_tensor(PCD0[:], PCD[0:48, :], PSB0[0:48, 1:2],
                                    op=OP.add)
            G0G = sb.tile([48, T], F32, tag="G0G")
            nc.vector.tensor_scalar_add(G0G[:], PK[0:48, C_EMG:C_EMG + T],
                                        PCD0[:])

            RH = sb.tile([112, 1], F32, tag="RH")
            nc.vector.tensor_reduce(RH[64:112, 0:1],
                                    PK[64:112, C_EMG:C_EMG + T],
                                    axis=AX.X, op=OP.add)
            RHp = sb.tile([112, 1], F32, tag="RHp")
            nc.vector.tensor_scalar(RHp[64:112, 0:1], PCD[64:112, :],
                                    float(T), RH[64:112, 0:1],
                                    op0=OP.mult, op1=OP.add)

            Z = sb.tile([112, 1], F32, tag="Z")
            nc.vector.scalar_tensor_tensor(
                Z[64:112, 0:1], PK[64:112, C_PRI:C_PRI + 1],
                PSB1[:, 0:1], RHp[64:112, 0:1],
                op0=OP.subtract, op1=OP.add)

            P0C = sb.tile([48, 1], F32, tag="P0C")
            nc.vector.tensor_tensor(P0C[:], PK[0:48, C_PRI:C_PRI + 1],
                                    PSB0[0:48, 0:1], op=OP.subtract)

            nc.tensor.transpose(ZT_p, Z[64:112, 0:1],
                                PK[64:112, C_ID + 64:C_ID + 112])
            ZTs = sb.tile([1, 48], F32, tag="ZTs")
            nc.vector.tensor_copy(ZTs[:], ZT_p)
            nc.tensor.matmul(ZR_p, ones48f, ZTs[:], start=True, stop=True)

            # ---- big pass: 6 chunks of (48, 8*64) ----------------------
            W = sb.tile([48, S], F32, tag="W")
            for c in range(NCHUNK):
                pc = ps.tile([48, CW], F32, tag=f"pp{c % 3}")
                f0 = F_EMF + c * CW
                for h in range(2):
                    o = h * HALF
                    nc.tensor.matmul(pc[:, o:o + HALF], ones48,
                                     FL[:, f0 + o:f0 + o + HALF],
                                     start=True, stop=False)
                    s1o = 64 + c * S1C + h * (S1C // 2)
                    nc.tensor.matmul(
                        pc[:, o:o + HALF], ones48,
                        PCTs[0:1, s1o:s1o + S1C // 2]
                        .unsqueeze(2).broadcast_to([1, S1C // 2, T]),
                        start=False, stop=True)

                D = sb.tile([48, CW], F32, tag=f"D{c % 3}")
                nc.vector.tensor_tensor(
                    D[:].rearrange("p (a b) -> p a b", a=S1C, b=T),
                    G0G[:].unsqueeze(1).broadcast_to([48, S1C, T]),
                    pc[:].rearrange("p (a b) -> p a b", a=S1C, b=T),
                    op=OP.subtract)
                U = sb.tile([48, CW], F32, tag=f"U{c % 3}")
                nc.scalar.activation(U[:], D[:], AF.Exp)
                V = sb.tile([48, CW], F32, tag=f"V{c % 3}")
                nc.scalar.activation(V[:], U[:], AF.Ln, bias=1.0)
                nc.vector.tensor_reduce(
                    W[:, c * S1C:(c + 1) * S1C],
                    V[:].rearrange("p (a b) -> p a b", a=S1C, b=T),
                    axis=AX.X, op=OP.add)

            # ---- final assembly + global LSE ---------------------------
            BETA = sb.tile([48, S], F32, tag="BETA")
            nc.vector.scalar_tensor_tensor(BETA[:], W[:], P0C[:], ZR_p,
                                           op0=OP.add, op1=OP.add)

            RS = sb.tile([48, 2], F32, tag="RS")        # [-rowmax | rowsum]
            nc.vector.tensor_reduce(RS[:, 0:1], BETA[:], axis=AX.X,
                                    op=OP.max, negate=True)
            EE = sb.tile([48, S], F32, tag="EE")
            nc.scalar.activation(EE[:], BETA[:], AF.Exp, bias=RS[:, 0:1],
                                 accum_out=RS[:, 1:2])

            # answer = M + ln(sum_r rowsum_r * exp(rowmax_r - M)), M = max rowmax
            LT2_p = SM[0:1, 320:368]
            nc.tensor.transpose(LT_p, RS[:, 0:1],
                                PK[0:48, C_ID:C_ID + 48])
            nc.tensor.transpose(LT2_p, RS[:, 1:2], PK[0:48, C_ID:C_ID + 48])
            LTs = sb.tile([1, 96], F32, tag="LTs")
            nc.vector.tensor_copy(LTs[:, 0:48], LT_p)
            nc.vector.tensor_copy(LTs[:, 48:96], LT2_p)

            NR = sb.tile([1, 48], F32, tag="NR")         # +rowmax
            nc.vector.tensor_scalar_mul(NR[:], LTs[:, 0:48], -1.0)
            GM = sb.tile([1, 1], F32, tag="GM")          # -max(rowmax) = -M
            nc.vector.tensor_reduce(GM[:], NR[:], axis=AX.X, op=OP.max,
                                    negate=True)
            EXG = sb.tile([1, 48], F32, tag="EXG")       # exp(rowmax - M)
            nc.scalar.activation(EXG[:], NR[:], AF.Exp, bias=GM[:])
            WS = sb.tile([1, 48], F32, tag="WS")
            nc.vector.tensor_mul(WS[:], EXG[:], LTs[:, 48:96])
            SF = sb.tile([1, 1], F32, tag="SF")
            nc.vector.tensor_reduce(SF[:], WS[:], axis=AX.X, op=OP.add)
            LF = sb.tile([1, 1], F32, tag="LF")
            nc.scalar.activation(LF[:], SF[:], AF.Ln)
            A0 = sb.tile([1, 1], F32, tag="A0")
            nc.vector.tensor_tensor(A0[:], LF[:], GM[:], op=OP.subtract)

            FIN = sb.tile([1, 1], F32, tag="FIN")
            nc.vector.tensor_scalar(FIN[:], CL[:, 1:2], float(T), A0[:],
                                    op0=OP.mult, op1=OP.add)
            nc.sync.dma_start(out_d[:, :], FIN[:])

    nc.compile()
    return nc


def _host_inputs(ys, transition, emission, choice, prior):
    ys = np.asarray(ys).astype(np.int64)
    tr = np.asarray(transition, np.float32)
    em = np.asarray(emission, np.float32)
    ch = np.asarray(choice, np.float32)
    pr = np.asarray(prior, np.float32)

    pk = np.zeros((128, PK_W), np.float32)
    pk[0:48, C_TT:C_TT + S] = tr[0]
    pk[64:112, C_TT:C_TT + S] = tr[1]
    pk[48:64, C_TT:C_EM + A] = NEG
    pk[112:128, C_TT:C_EM + A] = NEG
    pk[0:48, C_EM:C_EM + A] = em[0]
    pk[64:112, C_EM:C_EM + A] = em[1]
    pk[0:48, C_DG] = np.diagonal(tr[0])
    pk[64:112, C_DG] = np.diagonal(tr[1])
    pk[0:48, C_EMG:C_EMG + T] = em[0][:, ys]
    pk[64:112, C_EMG:C_EMG + T] = em[1][:, ys]
    pk[0:48, C_PRI] = pr[0]
    pk[64:112, C_PRI] = pr[1]
    pk[0:112, C_ID:C_ID + 112] = np.eye(112, dtype=np.float32)

    fl = np.zeros((1, FLAT_W), np.float32)
    fl[0, F_PRI0:F_PRI0 + S] = pr[0]
    fl[0, F_PRI1:F_PRI1 + S] = pr[1]
    fl[0, F_CH:F_CH + K] = ch
    fl[0, F_CH + K:144] = NEG
    fl[0, F_ONES:F_ONES + 48] = 1.0
    fl[0, F_EMF:F_EMF + S * T] = em[1][:, ys].reshape(-1)

    return {"pk": pk, "flat": fl}


def kernel(ys, transition, emission, choice, prior):
    global _CACHED_NC
    if _CACHED_NC is None:
        _CACHED_NC = _build_nc()
    in_map = _host_inputs(ys, transition, emission, choice, prior)
    in_maps = [dict(in_map) for _ in range(N_CORES)]
    res = run_bass_kernel_spmd(_CACHED_NC, in_maps,
                               core_ids=list(range(N_CORES)))
    return np.float32(res.results[0]["out"][0, 0]).reshape(())


# revision 13
# speedup vs baseline: 1.0291x; 1.0291x over previous
"""Trainium2 Bass kernel v2 for nn_InterleavedHiddenMarkovChain_47261820125822.

Same exact collapsed math as the baseline (see kernel.py docstring), but
re-engineered against the trace evidence from the 66us baseline:

 1. Exp/Ln ONLY (they share one activation table: `natural_log_exp_and_others`)
    -> one hidden ACT_TABLE_LOAD instead of 12 (-15.4us).
 2. Observation-gather formulation: the host gathers emis[:, ys] (pure int
    indexing -> "index prep"), so the time-sum becomes a plain innermost-axis
    tensor_reduce over t in a (48 x 48 x 64) view -- no fp32 PE reduction, no
    single-partition (1,2304) ops (-~29us).
 3. The per-s1 normalization term rides into PSUM via cheap rank-1 f32r
    matmuls pre-accumulated with the emission rows; the big pass is a
    6-chunk DVE->ACT->ACT->DVE pipeline over (48p, 512f) tiles.
 4. All partition-crossing is done by tiny PE transposes/rank-1 matmuls.

Math recap (K=2, S=48, A=64, T=64):
    ghat_c[s,a] = choice_l[c] + trans_l[c,s,s] + emis_l[c,s,a]
    beta[s0,s1] = prior_l[0,s0] + prior_l[1,s1]
                  + sum_t LSE(ghat_0[s0,y_t], ghat_1[s1,y_t])
    answer      = LSE_{s0,s1} beta     (+ LSE_c(choice_l) == 0 exactly)
with LSE(p,q) = q + ln1p(exp(p-q)):
    sum_t LSE(...) = R1h[s1] + W[s0,s1]
    R1h[s1] = sum_t emis[1,s1,y_t] + T*(PC1[s1] + cl1)
    W[s0,s1] = sum_t ln1p(exp(D[s0,s1,t]))
    D[s0,s1,t] = g0G[s0,t] - (emis[1,s1,y_t] + PC1[s1])
    g0G[s0,t] = emis[0,s0,y_t] + PC0[s0] + (cl0-cl1)
    PCc[s] = trans[c,s,s] - lseT[c,s] - lseE[c,s]   (c-row logsumexps)
"""

import numpy as np

import concourse.bass as bass
import concourse.bacc as bacc
import concourse.mybir as mybir
from concourse import tile
from concourse.bass_utils import run_bass_kernel_spmd

F32 = mybir.dt.float32
F32R = mybir.dt.float32r
AF = mybir.ActivationFunctionType
AX = mybir.AxisListType
OP = mybir.AluOpType

K, S, A, T = 2, 48, 64, 64
N_CORES = 8
NEG = -1.0e30

# PK column layout (128 partitions; c0 rows at 0:48, c1 rows at 64:112)
C_TT = 0          # 48: transition rows
C_EM = 48         # 64: emission rows
C_DG = 112        # 1:  transition diagonal
C_EMG = 113       # 64: emis[c][:, ys] gather
C_PRI = 177       # 1:  prior[c] as column
C_ID = 178        # 112: identity(112)
PK_W = 292

# FLAT row layout (single partition)
F_PRI0 = 0        # 48
F_PRI1 = 48       # 48
F_CH = 96         # 2 (then -1e30 pad to 144)
F_ONES = 144      # 48
F_EMF = 192       # 3072: emis[1, s1, ys[t]] at s1*64+t
FLAT_W = F_EMF + S * T   # 3264

NCHUNK = 6
CW = (S * T) // NCHUNK   # 512 free per chunk = 8 s1-values x 64 t
S1C = S // NCHUNK        # 8
HALF = CW               # psum-bank-sized matmul piece

_CACHED_NC = None

# The stock table-load pass serves Exp from `exp_and_others` and Ln from
# `natural_log` (first table containing each), reloading the activation
# table on every Exp<->Ln alternation (1283ns each, 13 loads in this
# kernel).  Both live in `natural_log_exp_and_others`; restrict the
# analysis (not the emitted ids -- dict order is preserved) so that is
# the only table serving them -> exactly one load, hoisted to the top.
_SHARED_TAB = "natural_log_exp_and_others"
_orig_get_tables = bacc.get_activation_tables


def _lnexp_tables(arch):
    out = {}
    for name, funcs in _orig_get_tables(arch).items():
        if name != _SHARED_TAB:
            funcs = funcs - {AF.Exp, AF.Ln}
        out[name] = funcs
    return out


bacc.get_activation_tables = _lnexp_tables


def _build_nc():
    nc = bacc.Bacc("TRN2", target_bir_lowering=False, debug=False)

    pk_d = nc.dram_tensor("pk", [128, PK_W], F32, kind="ExternalInput")
    fl_d = nc.dram_tensor("flat", [1, FLAT_W], F32R, kind="ExternalInput")
    out_d = nc.dram_tensor("out", [1, 1], F32, kind="ExternalOutput")

    with tile.TileContext(nc) as tc:
        with (
            tc.tile_pool(name="sb", bufs=1) as sb,
            tc.tile_pool(name="ps", bufs=1, space="PSUM") as ps,
        ):
            PK = sb.tile([128, PK_W], F32, tag="PK")
            nc.sync.dma_start(PK[:], pk_d[:, :])
            FL = sb.tile([1, FLAT_W], F32R, tag="FL")
            nc.sync.dma_start(FL[:], fl_d[:, :])

            ones48 = FL[:, F_ONES:F_ONES + 48]          # (1,48) f32r
            ones48f = ones48.bitcast(F32)

            # ---- batched row logsumexps --------------------------------
            NM = sb.tile([112, 8], F32, tag="NM")
            nc.vector.memset(NM[:], 0.0)
            SS = sb.tile([112, 8], F32, tag="SS")
            nc.vector.memset(SS[:], 1.0)

            nc.vector.tensor_reduce(NM[:, 0:1], PK[0:112, C_TT:C_EM + A],
                                    axis=AX.X, op=OP.max, negate=True)
            # prior0 | prior1 | choice(+pad) as 3x48 segments on partition 0
            nc.vector.tensor_reduce(
                NM[0:1, 2:5],
                FL[:, 0:144].bitcast(F32).rearrange("p (g w) -> p g w", g=3, w=48),
                axis=AX.X, op=OP.max, negate=True)

            # shared row max for the TT and EM segments (inputs are ~N(0,1),
            # so exp(x - m) cannot underflow for the smaller segment)
            EX = sb.tile([112, S + A], F32, tag="EX")
            nc.scalar.activation(EX[:], PK[0:112, C_TT:C_EM + A], AF.Exp,
                                 bias=NM[:, 0:1])
            nc.vector.tensor_reduce(SS[:, 0:1], EX[:, 0:S], axis=AX.X,
                                    op=OP.add)
            nc.vector.tensor_reduce(SS[:, 1:2], EX[:, S:S + A], axis=AX.X,
                                    op=OP.add)

            PRD = sb.tile([1, 144], F32, tag="PRD")
            nc.vector.tensor_tensor(
                PRD[:].rearrange("p (g w) -> p g w", g=3, w=48),
                FL[:, 0:144].bitcast(F32).rearrange("p (g w) -> p g w", g=3, w=48),
                NM[0:1, 2:5].unsqueeze(2).broadcast_to([1, 3, 48]),
                op=OP.add)
            E3 = sb.tile([1, 144], F32, tag="E3")
            nc.scalar.activation(E3[:], PRD[:], AF.Exp)
            nc.vector.tensor_reduce(
                SS[0:1, 2:5],
                E3[:].rearrange("p (g w) -> p g w", g=3, w=48),
                axis=AX.X, op=OP.add)

            LNS = sb.tile([112, 8], F32, tag="LNS")
            nc.scalar.activation(LNS[0:112, 0:5], SS[0:112, 0:5], AF.Ln)
            LSE = sb.tile([112, 8], F32, tag="LSE")
            nc.vector.tensor_tensor(LSE[0:1, 2:5], LNS[0:1, 2:5],
                                    NM[0:1, 2:5], op=OP.subtract)
            # LSE row0: 2=lseP0, 3=lseP1, 4=lseC

            # ---- per-row constants -------------------------------------
            # PCD = DG - lseT - lseE = DG - ln(S1) - ln(S2) + 2*nm
            T1 = sb.tile([112, 1], F32, tag="T1")
            nc.vector.tensor_tensor(T1[:], PK[0:112, C_DG:C_DG + 1],
                                    LNS[0:112, 0:1], op=OP.subtract)
            T2 = sb.tile([112, 1], F32, tag="T2")
            nc.vector.tensor_tensor(T2[:], T1[:], LNS[0:112, 1:2],
                                    op=OP.subtract)
            PCD = sb.tile([112, 1], F32, tag="PCD")
            nc.vector.tensor_scalar(PCD[:], NM[:, 0:1], 2.0, T2[:],
                                    op0=OP.mult, op1=OP.add)

            CL = sb.tile([1, 2], F32, tag="CL")
            nc.vector.tensor_scalar_sub(CL[:], FL[:, F_CH:F_CH + 2].bitcast(F32),
                                        LSE[0:1, 4:5])
            DCL = sb.tile([1, 1], F32, tag="DCL")
            nc.vector.tensor_tensor(DCL[:], CL[:, 0:1], CL[:, 1:2],
                                    op=OP.subtract)

            # ---- PE: transpose PCD -> row; rank-1 partition broadcasts --
            # all small PSUM tensors share one bank-sized tile (disjoint cols)
            SM = ps.tile([112, 512], F32, tag="sm")
            PCT_p = SM[0:1, 0:112]
            PSB0 = SM[0:48, 112:114]
            PSB1 = SM[64:112, 114:115]
            ZT_p = SM[0:1, 128:176]
            ZR_p = SM[0:48, 192:240]
            LT_p = SM[0:1, 256:304]
            nc.tensor.transpose(PCT_p, PCD[:], PK[0:112, C_ID:C_ID + 112])
            PCTs = sb.tile([1, 112], F32R, tag="PCTs")
            nc.vector.tensor_copy(PCTs[:], PCT_p)

            nc.tensor.matmul(PSB0[:, 0:1], ones48f, LSE[0:1, 2:3],
                             start=True, stop=True)
            nc.tensor.matmul(PSB0[:, 1:2], ones48f, DCL[:], start=True,
                             stop=True)
            nc.tensor.matmul(PSB1[:, 0:1], ones48f, LSE[0:1, 3:4],
                             start=True, stop=True)

            # ---- g0G and s1-side columns -------------------------------
            PCD0 = sb.tile([48, 1], F32, tag="PCD0")
            nc.vector.tensor_tensor(PCD0[:], PCD[0:48, :], PSB0[0:48, 1:2],
                                    op=OP.add)
            G0G = sb.tile([48, T], F32, tag="G0G")
            nc.vector.tensor_scalar_add(G0G[:], PK[0:48, C_EMG:C_EMG + T],
                                        PCD0[:])

            RH = sb.tile([112, 1], F32, tag="RH")
            nc.vector.tensor_reduce(RH[64:112, 0:1],
                                    PK[64:112, C_EMG:C_EMG + T],
                                    axis=AX.X, op=OP.add)
            RHp = sb.tile([112, 1], F32, tag="RHp")
            nc.vector.tensor_scalar(RHp[64:112, 0:1], PCD[64:112, :],
                                    float(T), RH[64:112, 0:1],
                                    op0=OP.mult, op1=OP.add)

            Z = sb.tile([112, 1], F32, tag="Z")
            nc.vector.scalar_tensor_tensor(
                Z[64:112, 0:1], PK[64:112, C_PRI:C_PRI + 1],
                PSB1[:, 0:1], RHp[64:112, 0:1],
                op0=OP.subtract, op1=OP.add)

            P0C = sb.tile([48, 1], F32, tag="P0C")
            nc.vector.tensor_tensor(P0C[:], PK[0:48, C_PRI:C_PRI + 1],
                                    PSB0[0:48, 0:1], op=OP.subtract)

            nc.tensor.transpose(ZT_p, Z[64:112, 0:1],
                                PK[64:112, C_ID + 64:C_ID + 112])
            ZTs = sb.tile([1, 48], F32, tag="ZTs")
            nc.vector.tensor_copy(ZTs[:], ZT_p)
            nc.tensor.matmul(ZR_p, ones48f, ZTs[:], start=True, stop=True)

            # ---- big pass: 6 chunks of (48, 8*64) ----------------------
            W = sb.tile([48, S], F32, tag="W")
            for c in range(NCHUNK):
                pc = ps.tile([48, CW], F32, tag=f"pp{c % 3}")
                f0 = F_EMF + c * CW
                nc.tensor.matmul(pc[:], ones48,
                                 FL[:, f0:f0 + CW],
                                 start=True, stop=False)
                nc.tensor.matmul(
                    pc[:], ones48,
                    PCTs[0:1, 64 + c * S1C:64 + (c + 1) * S1C]
                    .unsqueeze(2).broadcast_to([1, S1C, T]),
                    start=False, stop=True)

                D = sb.tile([48, CW], F32, tag=f"D{c % 3}")
                nc.vector.tensor_tensor(
                    D[:].rearrange("p (a b) -> p a b", a=S1C, b=T),
                    G0G[:].unsqueeze(1).broadcast_to([48, S1C, T]),
                    pc[:].rearrange("p (a b) -> p a b", a=S1C, b=T),
                    op=OP.subtract)
                U = sb.tile([48, CW], F32, tag=f"U{c % 3}")
                nc.scalar.activation(U[:], D[:], AF.Exp)
                V = sb.tile([48, CW], F32, tag=f"V{c % 3}")
                nc.scalar.activation(V[:], U[:], AF.Ln, bias=1.0)
                nc.vector.tensor_reduce(
                    W[:, c * S1C:(c + 1) * S1C],
                    V[:].rearrange("p (a b) -> p a b", a=S1C, b=T),
                    axis=AX.X, op=OP.add)

            # ---- final assembly + global LSE ---------------------------
            BETA = sb.tile([48, S], F32, tag="BETA")
            nc.vector.scalar_tensor_tensor(BETA[:], W[:], P0C[:], ZR_p,
                                           op0=OP.add, op1=OP.add)

            RS = sb.tile([48, 2], F32, tag="RS")        # [-rowmax | rowsum]
            nc.vector.tensor_reduce(RS[:, 0:1], BETA[:], axis=AX.X,
                                    op=OP.max, negate=True)
            EE = sb.tile([48, S], F32, tag="EE")
            nc.scalar.activation(EE[:], BETA[:], AF.Exp, bias=RS[:, 0:1],
                                 accum_out=RS[:, 1:2])

            # answer = M + ln(sum_r rowsum_r * exp(rowmax_r - M)), M = max rowmax
            LT2_p = SM[0:1, 320:368]
            nc.tensor.transpose(LT_p, RS[:, 0:1],
                                PK[0:48, C_ID:C_ID + 48])
            nc.tensor.transpose(LT2_p, RS[:, 1:2], PK[0:48, C_ID:C_ID + 48])
            LTs = sb.tile([1, 96], F32, tag="LTs")
            nc.vector.tensor_copy(LTs[:, 0:48], LT_p)
            nc.vector.tensor_copy(LTs[:, 48:96], LT2_p)

            NR = sb.tile([1, 48], F32, tag="NR")         # +rowmax
            nc.vector.tensor_scalar_mul(NR[:], LTs[:, 0:48], -1.0)
            GM = sb.tile([1, 1], F32, tag="GM")          # -max(rowmax) = -M
            nc.vector.tensor_reduce(GM[:], NR[:], axis=AX.X, op=OP.max,
                                    negate=True)
            EXG = sb.tile([1, 48], F32, tag="EXG")       # exp(rowmax - M)
            nc.scalar.activation(EXG[:], NR[:], AF.Exp, bias=GM[:])
            WS = sb.tile([1, 48], F32, tag="WS")
            nc.vector.tensor_mul(WS[:], EXG[:], LTs[:, 48:96])
            SF = sb.tile([1, 1], F32, tag="SF")
            nc.vector.tensor_reduce(SF[:], WS[:], axis=AX.X, op=OP.add)
            LF = sb.tile([1, 1], F32, tag="LF")
            nc.scalar.activation(LF[:], SF[:], AF.Ln)
            A0 = sb.tile([1, 1], F32, tag="A0")
            nc.vector.tensor_tensor(A0[:], LF[:], GM[:], op=OP.subtract)

            FIN = sb.tile([1, 1], F32, tag="FIN")
            nc.vector.tensor_scalar(FIN[:], CL[:, 1:2], float(T), A0[:],
                                    op0=OP.mult, op1=OP.add)
            nc.sync.dma_start(out_d[:, :], FIN[:])

    nc.compile()
    return nc


def _host_inputs(ys, transition, emission, choice, prior):
    ys = np.asarray(ys).astype(np.int64)
    tr = np.asarray(transition, np.float32)
    em = np.asarray(emission, np.float32)
    ch = np.asarray(choice, np.float32)
    pr = np.asarray(prior, np.float32)

    pk = np.zeros((128, PK_W), np.float32)
    pk[0:48, C_TT:C_TT + S] = tr[0]
    pk[64:112, C_TT:C_TT + S] = tr[1]
    pk[48:64, C_TT:C_EM + A] = NEG
    pk[112:128, C_TT:C_EM + A] = NEG
    pk[0:48, C_EM:C_EM + A] = em[0]
    pk[64:112, C_EM:C_EM + A] = em[1]
    pk[0:48, C_DG] = np.diagonal(tr[0])
    pk[64:112, C_DG] = np.diagonal(tr[1])
    pk[0:48, C_EMG:C_EMG + T] = em[0][:, ys]
    pk[64:112, C_EMG:C_EMG + T] = em[1][:, ys]
    pk[0:48, C_PRI] = pr[0]
    pk[64:112, C_PRI] = pr[1]
    pk[0:112, C_ID:C_ID + 112] = np.eye(112, dtype=np.float32)

    fl = np.zeros((1, FLAT_W), np.float32)
    fl[0, F_PRI0:F_PRI0 + S] = pr[0]
    fl[0, F_PRI1:F_PRI1 + S] = pr[1]
    fl[0, F_CH:F_CH + K] = ch
    fl[0, F_CH + K:144] = NEG
    fl[0, F_ONES:F_ONES + 48] = 1.0
    fl[0, F_EMF:F_EMF + S * T] = em[1][:, ys].reshape(-1)

    return {"pk": pk, "flat": fl}


def kernel(ys, transition, emission, choice, prior):
    global _CACHED_NC
    if _CACHED_NC is None:
        _CACHED_NC = _build_nc()
    in_map = _host_inputs(ys, transition, emission, choice, prior)
    in_maps = [dict(in_map) for _ in range(N_CORES)]
    res = run_bass_kernel_spmd(_CACHED_NC, in_maps,
                               core_ids=list(range(N_CORES)))
    return np.float32(res.results[0]["out"][0, 0]).reshape(())


# revision 14
# speedup vs baseline: 1.0432x; 1.0137x over previous
"""Trainium2 Bass kernel v2 for nn_InterleavedHiddenMarkovChain_47261820125822.

Same exact collapsed math as the baseline (see kernel.py docstring), but
re-engineered against the trace evidence from the 66us baseline:

 1. Exp/Ln ONLY (they share one activation table: `natural_log_exp_and_others`)
    -> one hidden ACT_TABLE_LOAD instead of 12 (-15.4us).
 2. Observation-gather formulation: the host gathers emis[:, ys] (pure int
    indexing -> "index prep"), so the time-sum becomes a plain innermost-axis
    tensor_reduce over t in a (48 x 48 x 64) view -- no fp32 PE reduction, no
    single-partition (1,2304) ops (-~29us).
 3. The per-s1 normalization term rides into PSUM via cheap rank-1 f32r
    matmuls pre-accumulated with the emission rows; the big pass is a
    6-chunk DVE->ACT->ACT->DVE pipeline over (48p, 512f) tiles.
 4. All partition-crossing is done by tiny PE transposes/rank-1 matmuls.

Math recap (K=2, S=48, A=64, T=64):
    ghat_c[s,a] = choice_l[c] + trans_l[c,s,s] + emis_l[c,s,a]
    beta[s0,s1] = prior_l[0,s0] + prior_l[1,s1]
                  + sum_t LSE(ghat_0[s0,y_t], ghat_1[s1,y_t])
    answer      = LSE_{s0,s1} beta     (+ LSE_c(choice_l) == 0 exactly)
with LSE(p,q) = q + ln1p(exp(p-q)):
    sum_t LSE(...) = R1h[s1] + W[s0,s1]
    R1h[s1] = sum_t emis[1,s1,y_t] + T*(PC1[s1] + cl1)
    W[s0,s1] = sum_t ln1p(exp(D[s0,s1,t]))
    D[s0,s1,t] = g0G[s0,t] - (emis[1,s1,y_t] + PC1[s1])
    g0G[s0,t] = emis[0,s0,y_t] + PC0[s0] + (cl0-cl1)
    PCc[s] = trans[c,s,s] - lseT[c,s] - lseE[c,s]   (c-row logsumexps)
"""

import numpy as np

import concourse.bass as bass
import concourse.bacc as bacc
import concourse.mybir as mybir
from concourse import tile
from concourse.bass_utils import run_bass_kernel_spmd

F32 = mybir.dt.float32
F32R = mybir.dt.float32r
AF = mybir.ActivationFunctionType
AX = mybir.AxisListType
OP = mybir.AluOpType

K, S, A, T = 2, 48, 64, 64
N_CORES = 8
NEG = -1.0e30

# PK column layout (128 partitions; c0 rows at 0:48, c1 rows at 64:112)
C_TT = 0          # 48: transition rows
C_EM = 48         # 64: emission rows
C_DG = 112        # 1:  transition diagonal
C_EMG = 113       # 64: emis[c][:, ys] gather
C_PRI = 177       # 1:  prior[c] as column
C_ID = 178        # 112: identity(112)
PK_W = 292

# FLAT row layout (single partition)
F_PRI0 = 0        # 48
F_PRI1 = 48       # 48
F_CH = 96         # 2 (then -1e30 pad to 144)
F_ONES = 144      # 48
F_EMF = 192       # 3072: emis[1, s1, ys[t]] at s1*64+t
FLAT_W = F_EMF + S * T   # 3264

NCHUNK = 6
CW = (S * T) // NCHUNK   # 512 free per chunk = 8 s1-values x 64 t
S1C = S // NCHUNK        # 8
HALF = CW               # psum-bank-sized matmul piece

_CACHED_NC = None

# The stock table-load pass serves Exp from `exp_and_others` and Ln from
# `natural_log` (first table containing each), reloading the activation
# table on every Exp<->Ln alternation (1283ns each, 13 loads in this
# kernel).  Both live in `natural_log_exp_and_others`; restrict the
# analysis (not the emitted ids -- dict order is preserved) so that is
# the only table serving them -> exactly one load, hoisted to the top.
_SHARED_TAB = "natural_log_exp_and_others"
_orig_get_tables = bacc.get_activation_tables


def _lnexp_tables(arch):
    out = {}
    for name, funcs in _orig_get_tables(arch).items():
        if name != _SHARED_TAB:
            funcs = funcs - {AF.Exp, AF.Ln}
        out[name] = funcs
    return out


bacc.get_activation_tables = _lnexp_tables


def _build_nc():
    nc = bacc.Bacc("TRN2", target_bir_lowering=False, debug=False)

    pk_d = nc.dram_tensor("pk", [128, PK_W], F32, kind="ExternalInput")
    fl_d = nc.dram_tensor("flat", [1, FLAT_W], F32R, kind="ExternalInput")
    out_d = nc.dram_tensor("out", [1, 1], F32, kind="ExternalOutput")

    with tile.TileContext(nc) as tc:
        with (
            tc.tile_pool(name="sb", bufs=1) as sb,
            tc.tile_pool(name="ps", bufs=1, space="PSUM") as ps,
        ):
            PK = sb.tile([128, PK_W], F32, tag="PK")
            nc.sync.dma_start(PK[:], pk_d[:, :])
            FL = sb.tile([1, FLAT_W], F32R, tag="FL")
            nc.sync.dma_start(FL[:], fl_d[:, :])

            ones48 = FL[:, F_ONES:F_ONES + 48]          # (1,48) f32r
            ones48f = ones48.bitcast(F32)

            # ---- batched row logsumexps --------------------------------
            NM = sb.tile([112, 8], F32, tag="NM")
            nc.vector.memset(NM[:], 0.0)
            SS = sb.tile([112, 8], F32, tag="SS")
            nc.vector.memset(SS[:], 1.0)

            nc.vector.tensor_reduce(NM[:, 0:1], PK[0:112, C_TT:C_TT + S],
                                    axis=AX.X, op=OP.max, negate=True)
            nc.vector.tensor_reduce(NM[:, 1:2], PK[0:112, C_EM:C_EM + A],
                                    axis=AX.X, op=OP.max, negate=True)
            # prior0 | prior1 | choice(+pad) as 3x48 segments on partition 0
            nc.vector.tensor_reduce(
                NM[0:1, 2:5],
                FL[:, 0:144].bitcast(F32).rearrange("p (g w) -> p g w", g=3, w=48),
                axis=AX.X, op=OP.max, negate=True)

            E1 = sb.tile([112, S], F32, tag="E1")
            nc.scalar.activation(E1[:], PK[0:112, C_TT:C_TT + S], AF.Exp,
                                 bias=NM[:, 0:1], accum_out=SS[:, 0:1])
            E2 = sb.tile([112, A], F32, tag="E2")
            nc.scalar.activation(E2[:], PK[0:112, C_EM:C_EM + A], AF.Exp,
                                 bias=NM[:, 1:2], accum_out=SS[:, 1:2])

            PRD = sb.tile([1, 144], F32, tag="PRD")
            nc.vector.tensor_tensor(
                PRD[:].rearrange("p (g w) -> p g w", g=3, w=48),
                FL[:, 0:144].bitcast(F32).rearrange("p (g w) -> p g w", g=3, w=48),
                NM[0:1, 2:5].unsqueeze(2).broadcast_to([1, 3, 48]),
                op=OP.add)
            E3 = sb.tile([1, 144], F32, tag="E3")
            nc.scalar.activation(E3[:], PRD[:], AF.Exp)
            nc.vector.tensor_reduce(
                SS[0:1, 2:5],
                E3[:].rearrange("p (g w) -> p g w", g=3, w=48),
                axis=AX.X, op=OP.add)

            LNS = sb.tile([112, 8], F32, tag="LNS")
            nc.scalar.activation(LNS[0:112, 0:5], SS[0:112, 0:5], AF.Ln)
            LSE = sb.tile([112, 8], F32, tag="LSE")
            nc.vector.tensor_tensor(LSE[0:112, 0:5], LNS[0:112, 0:5],
                                    NM[0:112, 0:5], op=OP.subtract)
            # LSE cols: 0=lse(trans row), 1=lse(emis row);
            # row0: 2=lseP0, 3=lseP1, 4=lseC

            # ---- per-row constants -------------------------------------
            T1 = sb.tile([112, 1], F32, tag="T1")
            nc.vector.tensor_tensor(T1[:], PK[0:112, C_DG:C_DG + 1],
                                    LSE[0:112, 0:1], op=OP.subtract)
            PCD = sb.tile([112, 1], F32, tag="PCD")
            nc.vector.tensor_tensor(PCD[:], T1[:], LSE[0:112, 1:2],
                                    op=OP.subtract)

            CL = sb.tile([1, 2], F32, tag="CL")
            nc.vector.tensor_scalar_sub(CL[:], FL[:, F_CH:F_CH + 2].bitcast(F32),
                                        LSE[0:1, 4:5])
            DCL = sb.tile([1, 1], F32, tag="DCL")
            nc.vector.tensor_tensor(DCL[:], CL[:, 0:1], CL[:, 1:2],
                                    op=OP.subtract)

            # ---- PE: transpose PCD -> row; rank-1 partition broadcasts --
            # all small PSUM tensors share one bank-sized tile (disjoint cols)
            SM = ps.tile([112, 512], F32, tag="sm")
            PCT_p = SM[0:1, 0:112]
            PSB0 = SM[0:48, 112:114]
            PSB1 = SM[64:112, 114:115]
            ZT_p = SM[0:1, 128:176]
            ZR_p = SM[0:48, 192:240]
            LT_p = SM[0:1, 256:304]
            nc.tensor.transpose(PCT_p, PCD[:], PK[0:112, C_ID:C_ID + 112])
            PCTs = sb.tile([1, 112], F32R, tag="PCTs")
            nc.vector.tensor_copy(PCTs[:], PCT_p)

            nc.tensor.matmul(PSB0[:, 0:1], ones48f, LSE[0:1, 2:3],
                             start=True, stop=True)
            nc.tensor.matmul(PSB0[:, 1:2], ones48f, DCL[:], start=True,
                             stop=True)
            nc.tensor.matmul(PSB1[:, 0:1], ones48f, LSE[0:1, 3:4],
                             start=True, stop=True)

            # ---- g0G and s1-side columns -------------------------------
            PCD0 = sb.tile([48, 1], F32, tag="PCD0")
            nc.vector.tensor_tensor(PCD0[:], PCD[0:48, :], PSB0[0:48, 1:2],
                                    op=OP.add)
            G0G = sb.tile([48, T], F32, tag="G0G")
            nc.vector.tensor_scalar_add(G0G[:], PK[0:48, C_EMG:C_EMG + T],
                                        PCD0[:])

            RH = sb.tile([112, 1], F32, tag="RH")
            nc.vector.tensor_reduce(RH[64:112, 0:1],
                                    PK[64:112, C_EMG:C_EMG + T],
                                    axis=AX.X, op=OP.add)
            RHp = sb.tile([112, 1], F32, tag="RHp")
            nc.vector.tensor_scalar(RHp[64:112, 0:1], PCD[64:112, :],
                                    float(T), RH[64:112, 0:1],
                                    op0=OP.mult, op1=OP.add)

            Z = sb.tile([112, 1], F32, tag="Z")
            nc.vector.scalar_tensor_tensor(
                Z[64:112, 0:1], PK[64:112, C_PRI:C_PRI + 1],
                PSB1[:, 0:1], RHp[64:112, 0:1],
                op0=OP.subtract, op1=OP.add)

            P0C = sb.tile([48, 1], F32, tag="P0C")
            nc.vector.tensor_tensor(P0C[:], PK[0:48, C_PRI:C_PRI + 1],
                                    PSB0[0:48, 0:1], op=OP.subtract)

            nc.tensor.transpose(ZT_p, Z[64:112, 0:1],
                                PK[64:112, C_ID + 64:C_ID + 112])
            ZTs = sb.tile([1, 48], F32, tag="ZTs")
            nc.vector.tensor_copy(ZTs[:], ZT_p)
            nc.tensor.matmul(ZR_p, ones48f, ZTs[:], start=True, stop=True)

            # ---- big pass: 6 chunks of (48, 8*64) ----------------------
            W = sb.tile([48, S], F32, tag="W")
            for c in range(NCHUNK):
                pc = ps.tile([48, CW], F32, tag=f"pp{c % 3}")
                f0 = F_EMF + c * CW
                nc.tensor.matmul(pc[:], ones48,
                                 FL[:, f0:f0 + CW],
                                 start=True, stop=False)
                nc.tensor.matmul(
                    pc[:], ones48,
                    PCTs[0:1, 64 + c * S1C:64 + (c + 1) * S1C]
                    .unsqueeze(2).broadcast_to([1, S1C, T]),
                    start=False, stop=True)

                D = sb.tile([48, CW], F32, tag=f"D{c % 3}")
                nc.vector.tensor_tensor(
                    D[:].rearrange("p (a b) -> p a b", a=S1C, b=T),
                    G0G[:].unsqueeze(1).broadcast_to([48, S1C, T]),
                    pc[:].rearrange("p (a b) -> p a b", a=S1C, b=T),
                    op=OP.subtract)
                U = sb.tile([48, CW], F32, tag=f"U{c % 3}")
                nc.scalar.activation(U[:], D[:], AF.Exp)
                V = sb.tile([48, CW], F32, tag=f"V{c % 3}")
                nc.scalar.activation(V[:], U[:], AF.Ln, bias=1.0)
                nc.vector.tensor_reduce(
                    W[:, c * S1C:(c + 1) * S1C],
                    V[:].rearrange("p (a b) -> p a b", a=S1C, b=T),
                    axis=AX.X, op=OP.add)

            # ---- final assembly + global LSE ---------------------------
            BETA = sb.tile([48, S], F32, tag="BETA")
            nc.vector.scalar_tensor_tensor(BETA[:], W[:], P0C[:], ZR_p,
                                           op0=OP.add, op1=OP.add)

            RS = sb.tile([48, 2], F32, tag="RS")        # [-rowmax | rowsum]
            nc.vector.tensor_reduce(RS[:, 0:1], BETA[:], axis=AX.X,
                                    op=OP.max, negate=True)
            EE = sb.tile([48, S], F32, tag="EE")
            nc.scalar.activation(EE[:], BETA[:], AF.Exp, bias=RS[:, 0:1],
                                 accum_out=RS[:, 1:2])

            # answer = M + ln(sum_r rowsum_r * exp(rowmax_r - M)), M = max rowmax
            LT2_p = SM[0:1, 320:368]
            nc.tensor.transpose(LT_p, RS[:, 0:1],
                                PK[0:48, C_ID:C_ID + 48])
            nc.tensor.transpose(LT2_p, RS[:, 1:2], PK[0:48, C_ID:C_ID + 48])
            LTs = sb.tile([1, 96], F32, tag="LTs")
            nc.vector.tensor_copy(LTs[:, 0:48], LT_p)
            nc.vector.tensor_copy(LTs[:, 48:96], LT2_p)

            NR = sb.tile([1, 48], F32, tag="NR")         # +rowmax
            nc.vector.tensor_scalar_mul(NR[:], LTs[:, 0:48], -1.0)
            GM = sb.tile([1, 1], F32, tag="GM")          # -max(rowmax) = -M
            nc.vector.tensor_reduce(GM[:], NR[:], axis=AX.X, op=OP.max,
                                    negate=True)
            EXG = sb.tile([1, 48], F32, tag="EXG")       # exp(rowmax - M)
            nc.scalar.activation(EXG[:], NR[:], AF.Exp, bias=GM[:])
            WS = sb.tile([1, 48], F32, tag="WS")
            nc.vector.tensor_mul(WS[:], EXG[:], LTs[:, 48:96])
            SF = sb.tile([1, 1], F32, tag="SF")
            nc.vector.tensor_reduce(SF[:], WS[:], axis=AX.X, op=OP.add)
            LF = sb.tile([1, 1], F32, tag="LF")
            nc.scalar.activation(LF[:], SF[:], AF.Ln)
            A0 = sb.tile([1, 1], F32, tag="A0")
            nc.vector.tensor_tensor(A0[:], LF[:], GM[:], op=OP.subtract)

            FIN = sb.tile([1, 1], F32, tag="FIN")
            nc.vector.tensor_scalar(FIN[:], CL[:, 1:2], float(T), A0[:],
                                    op0=OP.mult, op1=OP.add)
            nc.sync.dma_start(out_d[:, :], FIN[:])

    nc.compile()
    return nc


def _host_inputs(ys, transition, emission, choice, prior):
    ys = np.asarray(ys).astype(np.int64)
    tr = np.asarray(transition, np.float32)
    em = np.asarray(emission, np.float32)
    ch = np.asarray(choice, np.float32)
    pr = np.asarray(prior, np.float32)

    pk = np.zeros((128, PK_W), np.float32)
    pk[0:48, C_TT:C_TT + S] = tr[0]
    pk[64:112, C_TT:C_TT + S] = tr[1]
    pk[48:64, C_TT:C_EM + A] = NEG
    pk[112:128, C_TT:C_EM + A] = NEG
    pk[0:48, C_EM:C_EM + A] = em[0]
    pk[64:112, C_EM:C_EM + A] = em[1]
    pk[0:48, C_DG] = np.diagonal(tr[0])
    pk[64:112, C_DG] = np.diagonal(tr[1])
    pk[0:48, C_EMG:C_EMG + T] = em[0][:, ys]
    pk[64:112, C_EMG:C_EMG + T] = em[1][:, ys]
    pk[0:48, C_PRI] = pr[0]
    pk[64:112, C_PRI] = pr[1]
    pk[0:112, C_ID:C_ID + 112] = np.eye(112, dtype=np.float32)

    fl = np.zeros((1, FLAT_W), np.float32)
    fl[0, F_PRI0:F_PRI0 + S] = pr[0]
    fl[0, F_PRI1:F_PRI1 + S] = pr[1]
    fl[0, F_CH:F_CH + K] = ch
    fl[0, F_CH + K:144] = NEG
    fl[0, F_ONES:F_ONES + 48] = 1.0
    fl[0, F_EMF:F_EMF + S * T] = em[1][:, ys].reshape(-1)

    return {"pk": pk, "flat": fl}


def kernel(ys, transition, emission, choice, prior):
    global _CACHED_NC
    if _CACHED_NC is None:
        _CACHED_NC = _build_nc()
    in_map = _host_inputs(ys, transition, emission, choice, prior)
    in_maps = [dict(in_map) for _ in range(N_CORES)]
    res = run_bass_kernel_spmd(_CACHED_NC, in_maps,
                               core_ids=list(range(N_CORES)))
    return np.float32(res.results[0]["out"][0, 0]).reshape(())
